# revision 53
# baseline (speedup 1.0000x reference)
"""Trainium2 Bass kernel for windowed cross-attention block.

Reference computation (per token stream of B*T*H*W tokens, C=256):
  qn/kn/vn = LN(q/k/v); window partition (2,8,8) -> windows of N=128 tokens;
  8-head attention with relative-position bias; proj; x = v + proj_out;
  x = x + MLP(LN(x)) with exact GELU.

Sharding: data-parallel over (B=2) x (T/2=4) = 8 slabs of [2,64,64,256],
one per NeuronCore.  Windows never cross slab boundaries, so there is no
inter-core communication.

Per-core kernel: 64 windows (8x8 over H,W), each a [128 tokens, 256] tile.

V2: all matmul operands are bf16 (fp32 matmuls cost 4 cycles/row vs 1 for
bf16 on TRN2); PSUM accumulation stays fp32.  No tile_position / non-zero
base-partition matmul operands (the backend rejects them): per-head score
operands are produced by 128-col PE transposes followed by SBUF->SBUF
partition-shift DMAs that pack all 8 heads of q^T/k^T into [32, 1024]
base-0 tiles.
"""

import numpy as np
from contextlib import ExitStack

import concourse.bass as bass
import concourse.bacc as bacc
import concourse.tile as tile
from concourse import mybir
from concourse._compat import with_exitstack

FP = mybir.dt.float32
BF = mybir.dt.bfloat16
P = 128
C = 256
HEADS = 8
HD = 32
SCALE = HD ** -0.5
MLP_H = 512
EPS = 1e-5
WS = (2, 8, 8)
N_TOK = 128  # tokens per window
NWIN_H = 8
NWIN_W = 8

AF = mybir.ActivationFunctionType
ALU = mybir.AluOpType
IO_G = 16  # windows per supergroup


# ---------------------------------------------------------------- host helpers
def _rel_pos_index():
    coords = np.stack(
        np.meshgrid(np.arange(WS[0]), np.arange(WS[1]), np.arange(WS[2]),
                    indexing="ij"))
    cf = coords.reshape(3, -1)
    rel = cf[:, :, None] - cf[:, None, :]
    rel = rel.transpose(1, 2, 0).copy()
    rel[..., 0] += WS[0] - 1
    rel[..., 1] += WS[1] - 1
    rel[..., 2] += WS[2] - 1
    rel[..., 0] *= (2 * WS[1] - 1) * (2 * WS[2] - 1)
    rel[..., 1] *= (2 * WS[2] - 1)
    return rel.sum(-1)  # [N, N]


def _constb_layout():
    cols = {}
    off = 0
    for name, width in [("identity", P), ("biasT", HEADS * N_TOK),
                        ("wpT", 2 * C), ("mw1T", 2 * MLP_H),
                        ("mw2T", 4 * C), ("ones8", HEADS), ("mask4", 4)]:
        cols[name] = off
        off += width
    cols["total"] = off
    return cols


CONSTB_COLS = _constb_layout()


# ---------------------------------------------------------------- bass program
GELU_FUNC = [AF.Gelu]  # sim_test swaps this for a sim-supported function


@with_exitstack
def _body(ctx: ExitStack, tc: tile.TileContext, t):
    nc = tc.nc
    q_d, k_d, v_d = t["q"], t["k"], t["v"]
    out_d = t["out"]

    const = ctx.enter_context(tc.tile_pool(name="const", bufs=1))
    io = ctx.enter_context(tc.tile_pool(name="io", bufs=2))
    ln = ctx.enter_context(tc.tile_pool(name="ln", bufs=4))
    act = ctx.enter_context(tc.tile_pool(name="act", bufs=3))
    ps_t = ctx.enter_context(tc.tile_pool(name="ps_t", bufs=2, space="PSUM"))
    ps_st = ctx.enter_context(tc.tile_pool(name="ps_st", bufs=1, space="PSUM"))
    ps_o = ctx.enter_context(tc.tile_pool(name="ps_o", bufs=1, space="PSUM"))
    ps_yf = ctx.enter_context(tc.tile_pool(name="ps_yf", bufs=2, space="PSUM"))
    ps_h = ctx.enter_context(tc.tile_pool(name="ps_h", bufs=1, space="PSUM"))

    # --- constants: one fp32 tile (eps) + one bf16 tile (everything else),
    # each loaded by a single DMA.
    cb = CONSTB_COLS
    constsb = const.tile([P, cb["total"]], BF)
    nc.gpsimd.dma_start(out=constsb, in_=t["constsb"][:, :])
    identb = constsb[:, cb["identity"]:cb["identity"] + P]
    biasTb = constsb[:, cb["biasT"]:cb["biasT"] + HEADS * N_TOK]
    wpT = constsb[:, cb["wpT"]:cb["wpT"] + 2 * C]
    mw1T = constsb[:, cb["mw1T"]:cb["mw1T"] + 2 * MLP_H]
    mw2T = constsb[:, cb["mw2T"]:cb["mw2T"] + 4 * C]
    ones8b = constsb[:, cb["ones8"]:cb["ones8"] + HEADS]
    mask4b = constsb[:, cb["mask4"]:cb["mask4"] + 4]
    constsf = const.tile([P, 1], FP)
    nc.gpsimd.dma_start(out=constsf, in_=t["constsf"][:, :])
    eps_t = constsf[:, 0:1]

    def pe_transpose(dst_psum_ap, src_sbuf_ap):
        nc.tensor.transpose(dst_psum_ap, src_sbuf_ap, identb)

    NW = NWIN_H * NWIN_W
    G = min(IO_G, NW)  # windows per supergroup (two-pass: attention, MLP)
    for g in range(NW // G):
        # ---- one DMA per tensor for G windows (cast to bf16 in-flight)
        q_g = io.tile([P, G * C], FP, tag="q_g")
        k_g = io.tile([P, G * C], FP, tag="k_g")
        v_g = io.tile([P, G * C], FP, tag="v_g")
        out_g = io.tile([P, G * C], FP, tag="out_g")
        xbuf = io.tile([P, G * C], FP, tag="xbuf")
        w0 = g * G
        src = lambda d: d[w0 * P:(w0 + G) * P, :].rearrange(
            "(w p) c -> p w c", w=G)
        dst3 = lambda tl: tl.rearrange("p (w c) -> p w c", w=G)
        nc.sync.dma_start(out=dst3(q_g), in_=src(q_d))
        nc.sync.dma_start(out=dst3(k_g), in_=src(k_d))
        nc.sync.dma_start(out=dst3(v_g), in_=src(v_d))

        # ---- batched LN stats + scales for the whole group.  One ACT sqrt
        # + one DVE reciprocal for all 3*G variances, so the ACT table only
        # switches set at pass boundaries (sqrt -> exp -> sqrt -> gelu).
        mvall = io.tile([P, 3 * G * 2], FP, tag="mvall")
        for ti, src_g in enumerate((q_g, k_g, v_g)):
            stats_g = ln.tile([P, G * 6], FP, tag=f"stats{ti}")
            for w in range(G):
                nc.vector.bn_stats(out=stats_g[:, 6 * w:6 * w + 6],
                                   in_=src_g[:, w * C:(w + 1) * C])
                nc.vector.bn_aggr(
                    out=mvall[:, ti * 2 * G + 2 * w:ti * 2 * G + 2 * w + 2],
                    in_=stats_g[:, 6 * w:6 * w + 6])
        mv3 = mvall.rearrange("p (t w s) -> p t w s", t=3, s=2)
        rsall = io.tile([P, 3 * G], FP, tag="rsall")
        rs3d = rsall.rearrange("p (t w) -> p t w", t=3)
        nc.scalar.activation(out=rs3d, in_=mv3[:, :, :, 1],
                             func=AF.Sqrt, bias=eps_t)
        nc.vector.reciprocal(out=rsall, in_=rsall)
        # fold the attention 1/sqrt(hd) scale into q's LN scale
        nc.vector.tensor_scalar_mul(out=rsall[:, 0:G], in0=rsall[:, 0:G],
                                    scalar1=float(SCALE))

        # ================= PASS A: attention; ACT runs only Exp =========
        for wl in range(G):
            cs = wl * C
            q_t = q_g[:, cs:cs + C]
            k_t = k_g[:, cs:cs + C]
            v_t = v_g[:, cs:cs + C]
            mean_c = lambda ti: mv3[:, ti, wl, 0:1]
            rs_c = lambda ti: rs3d[:, ti, wl:wl + 1]

            # LN applies (bf16 out): q on DVE, k/v on gpsimd.
            # high_priority pulls this block (which feeds the next scores
            # matmuls) ahead of the previous window's exp in engine queues.
            prio = ctx_prio = tc.high_priority(offset=200)
            ctx_prio.__enter__()
            qn = act.tile([P, C], BF, tag="qn")
            kn = act.tile([P, C], BF, tag="kn")
            vn33 = act.tile([P, HEADS * 33], BF, tag="vn33")
            vn3d = vn33.rearrange("p (h x) -> p h x", x=33)
            nc.vector.tensor_scalar(
                out=qn, in0=q_t, scalar1=mean_c(0), scalar2=rs_c(0),
                op0=ALU.subtract, op1=ALU.mult)
            nc.gpsimd.tensor_scalar(
                out=kn, in0=k_t, scalar1=mean_c(1), scalar2=rs_c(1),
                op0=ALU.subtract, op1=ALU.mult)
            nc.gpsimd.tensor_scalar(
                out=vn3d[:, :, 0:32], in0=v_t, scalar1=mean_c(2),
                scalar2=rs_c(2), op0=ALU.subtract, op1=ALU.mult)
            nc.gpsimd.tensor_copy(
                out=vn3d[:, :, 32:33].rearrange("p h x -> p (h x)"), in_=ones8b)

            # transpose qn, kn; copy q^T out on ACT; k^T feeds the masked
            # per-head copies straight from PSUM on DVE
            ptqk = ps_t.tile([P, 4 * P], BF, tag="pt")
            pe_transpose(ptqk[:, 0:P], qn[:, 0:P])
            pe_transpose(ptqk[:, P:2 * P], qn[:, P:2 * P])
            pe_transpose(ptqk[:, 2 * P:3 * P], kn[:, 0:P])
            pe_transpose(ptqk[:, 3 * P:4 * P], kn[:, 3 * P - 2 * P:4 * P - 2 * P])
            qkT = act.tile([P, 4 * P], BF, tag="qkT")
            nc.scalar.copy(out=qkT, in_=ptqk)
            qT = qkT[:, 0:2 * P]
            kTm = act.tile([P, HEADS * P], BF, tag="kTm")
            in0 = qkT[:, 2 * P:4 * P].rearrange("p (c t) -> p c t", c=2) \
                .unsqueeze(2).broadcast_to([P, 2, 4, P])
            in1 = mask4b.unsqueeze(1).unsqueeze(3).broadcast_to([P, 2, 4, P])
            nc.gpsimd.tensor_tensor(
                out=kTm.rearrange("p (c j t) -> p c j t", c=2, j=4),
                in0=in0, in1=in1, op=ALU.mult)
            ctx_prio.__exit__(None, None, None)

            # scores^T[m, (h n)] = bias^T + k^T . q  (PSUM fp32)
            st = ps_st.tile([P, HEADS * N_TOK], FP)
            for bank in range(2):
                nc.tensor.matmul(
                    st[:, bank * 512:(bank + 1) * 512], identb,
                    biasTb[:, bank * 512:(bank + 1) * 512],
                    start=True, stop=False)
            for h in range(HEADS):
                nc.tensor.matmul(
                    st[:, h * N_TOK:(h + 1) * N_TOK],
                    kTm[:, h * P:(h + 1) * P],
                    qT[:, (h // 4) * P:(h // 4 + 1) * P],
                    start=False, stop=(h % 4 == 3))

            # exp (no max subtraction: |scores| is small by construction)
            ptile = act.tile([P, HEADS * N_TOK], BF, tag="ptile")
            nc.scalar.activation(out=ptile, in_=st, func=AF.Exp)

            # out33[n, (h,33)] = P~^T.T @ [vhat_h | 1]
            o33 = ps_o.tile([P, HEADS * 33], FP)
            for h in range(HEADS):
                nc.tensor.matmul(
                    o33[:, h * 33:(h + 1) * 33],
                    ptile[:, h * N_TOK:(h + 1) * N_TOK],
                    vn33[:, h * 33:(h + 1) * 33],
                    start=(h == 0), stop=(h == HEADS - 1))

            # normalize by the ones-column sums -> A[tok, C] (bf16)
            o3d = o33.rearrange("p (h x) -> p h x", x=33)
            rs_a = ln.tile([P, HEADS], FP, tag="rs_a")
            nc.vector.reciprocal(
                out=rs_a, in_=o3d[:, :, 32:33].rearrange("p h x -> p (h x)"))
            a_t = act.tile([P, C], BF, tag="a_t")
            nc.vector.tensor_tensor(
                out=a_t.rearrange("p (h d) -> p h d", h=HEADS),
                in0=o3d[:, :, 0:32],
                in1=rs_a.unsqueeze(2).broadcast_to([P, HEADS, HD]),
                op=ALU.mult)

            # proj: Y[tok, co] = A @ Wp^T ; lhsT = A^T
            pta = ps_t.tile([P, 4 * P], BF, tag="pt")
            pe_transpose(pta[:, 0:P], a_t[:, 0:P])
            pe_transpose(pta[:, P:2 * P], a_t[:, P:2 * P])
            aT = act.tile([P, C], BF, tag="aT")
            nc.scalar.copy(out=aT, in_=pta[:, 0:2 * P])
            y_ps = ps_yf.tile([P, C], FP, tag="yf")
            for ct in range(2):
                nc.tensor.matmul(
                    y_ps, aT[:, ct * P:(ct + 1) * P],
                    wpT[:, ct * C:(ct + 1) * C],
                    start=(ct == 0), stop=(ct == 1))

            # residual 1 into the supergroup x buffer (fp32)
            nc.vector.tensor_tensor(out=xbuf[:, cs:cs + C], in0=v_t, in1=y_ps,
                                    op=ALU.add)

        # ---- batched LN2 stats + scales (one sqrt, one recip)
        mv2all = io.tile([P, G * 2], FP, tag="mv2all")
        for w in range(G):
            stats2 = ln.tile([P, 6], FP, tag="stats2")
            nc.vector.bn_stats(out=stats2, in_=xbuf[:, w * C:(w + 1) * C])
            nc.vector.bn_aggr(out=mv2all[:, 2 * w:2 * w + 2], in_=stats2)
        rs2all = io.tile([P, G], FP, tag="rs2all")
        nc.scalar.activation(
            out=rs2all, in_=mv2all.rearrange("p (w s) -> p w s", s=2)[:, :, 1],
            func=AF.Sqrt, bias=eps_t)
        nc.vector.reciprocal(out=rs2all, in_=rs2all)

        # ================= PASS B: MLP; ACT runs only Gelu ==============
        for wl in range(G):
            cs = wl * C
            x_t = xbuf[:, cs:cs + C]
            xn = act.tile([P, C], BF, tag="xn")
            nc.vector.tensor_scalar(
                out=xn, in0=x_t, scalar1=mv2all[:, 2 * wl:2 * wl + 1],
                scalar2=rs2all[:, wl:wl + 1],
                op0=ALU.subtract, op1=ALU.mult)
            ptx = ps_t.tile([P, 4 * P], BF, tag="pt")
            pe_transpose(ptx[:, 0:P], xn[:, 0:P])
            pe_transpose(ptx[:, P:2 * P], xn[:, P:2 * P])
            xnT = act.tile([P, C], BF, tag="xnT")
            nc.scalar.copy(out=xnT, in_=ptx[:, 0:2 * P])
            h1_ps = ps_h.tile([P, MLP_H], FP)
            for ct in range(2):
                nc.tensor.matmul(
                    h1_ps, xnT[:, ct * P:(ct + 1) * P],
                    mw1T[:, ct * MLP_H:(ct + 1) * MLP_H],
                    start=(ct == 0), stop=(ct == 1))
            h1 = act.tile([P, MLP_H], BF, tag="h1")
            nc.scalar.activation(out=h1, in_=h1_ps, func=GELU_FUNC[0])

            pth = ps_t.tile([P, 4 * P], BF, tag="pt")
            for ht in range(4):
                pe_transpose(pth[:, ht * P:(ht + 1) * P], h1[:, ht * P:(ht + 1) * P])
            h1T = act.tile([P, MLP_H], BF, tag="h1T")
            nc.vector.tensor_copy(out=h1T[:, 0:2 * P], in_=pth[:, 0:2 * P])
            nc.scalar.copy(out=h1T[:, 2 * P:4 * P], in_=pth[:, 2 * P:4 * P])
            o2_ps = ps_yf.tile([P, C], FP, tag="yf")
            for ht in range(4):
                nc.tensor.matmul(
                    o2_ps, h1T[:, ht * P:(ht + 1) * P],
                    mw2T[:, ht * C:(ht + 1) * C],
                    start=(ht == 0), stop=(ht == 3))

            # residual 2 into the group store tile
            nc.vector.tensor_tensor(out=out_g[:, cs:cs + C], in0=x_t, in1=o2_ps,
                                    op=ALU.add)

        nc.sync.dma_start(
            out=out_d[w0 * P:(w0 + G) * P, :].rearrange("(w p) c -> p w c", w=G),
            in_=out_g.rearrange("p (w c) -> p w c", w=G))


def build_nc():
    nc = bacc.Bacc()
    t = {}
    t["q"] = nc.dram_tensor("q", [64 * P, C], FP, kind="ExternalInput")[:]
    t["k"] = nc.dram_tensor("k", [64 * P, C], FP, kind="ExternalInput")[:]
    t["v"] = nc.dram_tensor("v", [64 * P, C], FP, kind="ExternalInput")[:]
    t["constsb"] = nc.dram_tensor("constsb", [P, CONSTB_COLS["total"]], BF,
                                  kind="ExternalInput")[:]
    t["constsf"] = nc.dram_tensor("constsf", [P, 1], FP, kind="ExternalInput")[:]
    t["out"] = nc.dram_tensor("out", [64 * P, C], FP, kind="ExternalOutput")[:]
    with tile.TileContext(nc) as tc:
        _body(tc, t)
    nc.compile()
    return nc


_NC_CACHE = None


def _get_nc():
    global _NC_CACHE
    if _NC_CACHE is None:
        _NC_CACHE = build_nc()
    return _NC_CACHE


def _host_prep(inputs):
    """Build the per-core shared (replicated) input arrays."""
    f32 = np.float32
    bf16 = mybir.dt.np(BF)
    bias_table = np.asarray(inputs["bias_table"], f32)
    rel = _rel_pos_index()
    bias_full = bias_table[rel]                      # [n, m, heads]
    biasT = np.ascontiguousarray(
        bias_full.transpose(1, 2, 0).reshape(P, HEADS * N_TOK))  # [m, (h n)]

    proj_w = np.asarray(inputs["proj_w"], f32)       # [co, ci]
    wpT = np.ascontiguousarray(
        proj_w.T.reshape(2, P, C).transpose(1, 0, 2).reshape(P, 2 * C))
    mw1 = np.asarray(inputs["mw1"], f32)             # [hid, ci]
    mw1T = np.ascontiguousarray(
        mw1.T.reshape(2, P, MLP_H).transpose(1, 0, 2).reshape(P, 2 * MLP_H))
    mw2 = np.asarray(inputs["mw2"], f32)             # [co, hid]
    mw2T = np.ascontiguousarray(
        mw2.T.reshape(4, P, C).transpose(1, 0, 2).reshape(P, 4 * C))

    # This kernel exploits that the affine LN params / biases in this problem
    # are identity (gamma=1, beta=0, linear biases=0).  Verify that.
    for name in ("gq", "gk", "gv", "g2"):
        assert np.allclose(np.asarray(inputs[name]), 1.0), f"{name} != 1"
    for name in ("bq", "bk", "bv", "b2", "proj_b", "mb1", "mb2"):
        assert np.allclose(np.asarray(inputs[name]), 0.0), f"{name} != 0"

    cb = CONSTB_COLS
    constsb = np.zeros((P, cb["total"]), np.float32)
    constsb[:, cb["identity"]:cb["identity"] + P] = np.eye(P, dtype=np.float32)
    constsb[:, cb["biasT"]:cb["biasT"] + HEADS * N_TOK] = biasT
    constsb[:, cb["wpT"]:cb["wpT"] + 2 * C] = wpT
    constsb[:, cb["mw1T"]:cb["mw1T"] + 2 * MLP_H] = mw1T
    constsb[:, cb["mw2T"]:cb["mw2T"] + 4 * C] = mw2T
    constsb[:, cb["ones8"]:cb["ones8"] + HEADS] = 1.0
    for j in range(4):
        constsb[32 * j:32 * (j + 1), cb["mask4"] + j] = 1.0
    constsf = np.full((P, 1), EPS, np.float32)
    return {"constsb": constsb.astype(bf16), "constsf": constsf}


def to_windows(slab):
    """[2, 64, 64, 256] -> [64*128, 256] window-major (hb, wb, t, hr, wr)."""
    x = slab.reshape(2, 8, 8, 8, 8, C)           # t, hb, hr, wb, wr, c
    x = x.transpose(1, 3, 0, 2, 4, 5)            # hb, wb, t, hr, wr, c
    return np.ascontiguousarray(x.reshape(64 * P, C))


def from_windows(wins):
    """[64*128, 256] window-major -> [2, 64, 64, 256]."""
    x = wins.reshape(8, 8, 2, 8, 8, C)           # hb, wb, t, hr, wr, c
    x = x.transpose(2, 0, 3, 1, 4, 5)            # t, hb, hr, wb, wr, c
    return x.reshape(2, 64, 64, C)


def _jax_fallback(inputs):
    """Data-parallel jax fallback (8 devices) if the bass path fails."""
    import jax

    q = np.asarray(inputs["q"], np.float32)
    k = np.asarray(inputs["k"], np.float32)
    v = np.asarray(inputs["v"], np.float32)
    devs = jax.devices()[:8]
    outs = []
    fn = None
    for core in range(8):
        b, t0 = core // 4, 2 * (core % 4)
        args = (q[b, t0:t0 + 2], k[b, t0:t0 + 2], v[b, t0:t0 + 2])
        args = tuple(jax.device_put(a, devs[core]) for a in args)
        if fn is None:
            fn = jax.jit(lambda qq, kk, vv: _reference_block(qq, kk, vv, inputs))
        outs.append(fn(*args))
    out = np.empty((2, 8, 64, 64, C), np.float32)
    for core in range(8):
        b, t0 = core // 4, 2 * (core % 4)
        out[b, t0:t0 + 2] = np.asarray(outs[core])
    return out


def _reference_block(q, k, v, inputs):
    import jax
    import jax.numpy as jnp
    DIM, HDIM = C, HD
    eps = EPS

    def layernorm(x, g, b):
        m = jnp.mean(x, axis=-1, keepdims=True)
        var = jnp.mean(jnp.square(x - m), axis=-1, keepdims=True)
        return (x - m) * jax.lax.rsqrt(var + eps) * g + b

    T, H, W = q.shape[0], q.shape[1], q.shape[2]
    shortcut = v
    qn = layernorm(q, np.asarray(inputs["gq"]), np.asarray(inputs["bq"]))
    kn = layernorm(k, np.asarray(inputs["gk"]), np.asarray(inputs["bk"]))
    vn = layernorm(v, np.asarray(inputs["gv"]), np.asarray(inputs["bv"]))

    def part(x):
        xx = x.reshape(T // 2, 2, H // 8, 8, W // 8, 8, DIM)
        xx = xx.transpose(0, 2, 4, 1, 3, 5, 6)
        return xx.reshape(-1, 128, DIM)

    qw, kw, vw = part(qn), part(kn), part(vn)
    B_ = qw.shape[0]
    th = lambda t: t.reshape(B_, 128, HEADS, HDIM).transpose(0, 2, 1, 3)
    qh, kh, vh = th(qw), th(kw), th(vw)
    attn = jnp.einsum('bhnd,bhmd->bhnm', qh * SCALE, kh)
    bias = np.asarray(inputs["bias_table"])[_rel_pos_index()]
    attn = attn + bias.transpose(2, 0, 1)[None]
    attn = jax.nn.softmax(attn, axis=-1)
    out = jnp.einsum('bhnm,bhmd->bhnd', attn, vh)
    out = out.transpose(0, 2, 1, 3).reshape(B_, 128, DIM)
    out = out @ np.asarray(inputs["proj_w"]).T + np.asarray(inputs["proj_b"])
    x = out.reshape(T // 2, H // 8, W // 8, 2, 8, 8, DIM)
    x = x.transpose(0, 3, 1, 4, 2, 5, 6).reshape(T, H, W, DIM)
    x = shortcut + x
    h1 = layernorm(x, np.asarray(inputs["g2"]), np.asarray(inputs["b2"]))
    h1 = jax.nn.gelu(h1 @ np.asarray(inputs["mw1"]).T + np.asarray(inputs["mb1"]),
                     approximate=False)
    x = x + (h1 @ np.asarray(inputs["mw2"]).T + np.asarray(inputs["mb2"]))
    return x


def kernel(**inputs):
    try:
        return _bass_kernel(**inputs)
    except Exception as e:  # pragma: no cover - safety net
        import traceback
        traceback.print_exc()
        print(f"bass path failed ({type(e).__name__}); using jax fallback",
              flush=True)
        return _jax_fallback(inputs)


def _bass_kernel(**inputs):
    from concourse.bass_utils import run_bass_kernel_spmd

    q = np.asarray(inputs["q"], np.float32)
    k = np.asarray(inputs["k"], np.float32)
    v = np.asarray(inputs["v"], np.float32)
    B, T, H, W, _ = q.shape
    shared = _host_prep(inputs)

    in_maps = []
    for core in range(8):
        b, t0 = core // 4, 2 * (core % 4)
        m = {
            "q": to_windows(q[b, t0:t0 + 2]),
            "k": to_windows(k[b, t0:t0 + 2]),
            "v": to_windows(v[b, t0:t0 + 2]),
        }
        m.update(shared)
        in_maps.append(m)

    nc = _get_nc()
    res = run_bass_kernel_spmd(nc, in_maps, list(range(8)))
    out = np.empty((B, T, H, W, C), np.float32)
    for core in range(8):
        b, t0 = core // 4, 2 * (core % 4)
        out[b, t0:t0 + 2] = from_windows(res.results[core]["out"])
    return out
